# revision 6
# baseline (speedup 1.0000x reference)
"""DeltaMPredictor Trainium2 kernel (8 NeuronCores, data-parallel over batch).

Pipeline per token (b, c):
    reg = thumb @ proj_w.T + proj_b            [2048] -> [512]
    y   = (reg - mean) * rstd                  per-camera LayerNorm (gamma/beta
                                               folded into the SwiGLU weights)
    gate = y @ (w_gate*gamma).T + w_gate@beta
    val  = y @ (w_val *gamma).T + w_val @beta
    h   = silu(gate) * val
    A   = reshape(h @ w_out.T, 6, 6); A -= A.T; clip frob to 3
    dM  = expm(A)  (Horner degree-6 Taylor + 5 squarings, batched on DVE)

Sharding: batch B=16384 split 8 ways (2048 rows/core); all weights replicated.
Per core the loop is camera-major (4 cameras x 4 tiles of 512 tokens).

Matmuls run in float32r (TF32-like, 1 cyc/row at N>=512) except the final
36-wide projection which is plain fp32. The thumbnail/y operands are
transposed on the PE (fp32 has no DMA transpose).
"""

import os
import sys

sys.path.insert(0, "/opt/trn_rl_repo")

from contextlib import ExitStack

import numpy as np

import concourse.bacc as bacc
import concourse.bass as bass
import concourse.tile as tile
from concourse import mybir
from concourse.bass_utils import run_bass_kernel_spmd
from concourse.masks import make_identity

B, C, D_BB, D = 16384, 4, 2048, 512
N_CORES = 8
BLOC = B // N_CORES          # 2048 batch rows per core
TOK = 512                    # tokens per tile
NT = BLOC // TOK             # 4 tiles per camera
G = TOK // 128               # 4 token chunks of 128 per tile
KE = D_BB // 128             # 16 contraction chunks for mm1
KD = D // 128                # 4 contraction chunks for mm2/mm3
MAX_NORM = 3.0
LN_EPS = 1e-5
EXP_S = 5                    # squarings in expm

F32 = mybir.dt.float32
F32R = mybir.dt.float32r
I32 = mybir.dt.int32
AL = mybir.AluOpType
AF = mybir.ActivationFunctionType
AX = mybir.AxisListType

_BUILD_CACHE = {}
last_results = None          # test harness introspection
last_in_maps = None


def _emit_rsqrt(nc, pool, out, x, n, tag, iters=2):
    """out = 1/sqrt(x) elementwise for [128, n] fp32 SBUF tiles.

    Magic-constant seed + `iters` Newton steps (rel err ~4e-6 at 2 iters).
    Safe for x == 0 (result is finite-huge, no NaN).
    """
    magic = pool.tile([128, 1], I32, tag=f"{tag}_magic")
    nc.vector.memset(magic, 0x5F3759DF)
    sh = pool.tile([128, n], I32, tag=f"{tag}_sh")
    nc.vector.tensor_scalar(
        out=sh, in0=x.bitcast(I32), scalar1=1, scalar2=None,
        op0=AL.logical_shift_right,
    )
    nc.vector.tensor_tensor(
        out=out.bitcast(I32),
        in0=magic[:, 0:1].broadcast_to((128, n)),
        in1=sh,
        op=AL.subtract,
    )
    tmp = pool.tile([128, n], F32, tag=f"{tag}_tmp")
    for _ in range(iters):
        nc.vector.tensor_tensor(out=tmp, in0=x, in1=out, op=AL.mult)
        nc.vector.tensor_tensor(out=tmp, in0=tmp, in1=out, op=AL.mult)
        nc.vector.tensor_scalar(
            out=tmp, in0=tmp, scalar1=-0.5, scalar2=1.5, op0=AL.mult, op1=AL.add
        )
        nc.vector.tensor_tensor(out=out, in0=out, in1=tmp, op=AL.mult)


def _emit_expm(nc, pool, A0):
    """A0 [128, G*36] fp32: skew, frob-clip, expm. Returns E [128, G*36]."""

    def v4(t):
        return t[:, :].rearrange("p (g i j) -> p g i j", g=G, i=6, j=6)

    S = pool.tile([128, G * 36], F32, tag="xS")
    nc.vector.tensor_tensor(
        out=v4(S), in0=v4(A0), in1=v4(A0).transpose([0, 1, 3, 2]), op=AL.subtract
    )
    SQ = pool.tile([128, G * 36], F32, tag="xSQ")
    nc.vector.tensor_tensor(out=SQ[:, :], in0=S[:, :], in1=S[:, :], op=AL.mult)
    ss = pool.tile([128, G], F32, tag="xss")
    nc.vector.tensor_reduce(
        out=ss[:, :],
        in_=SQ[:, :].rearrange("p (g a) -> p g a", g=G),
        axis=AX.X,
        op=AL.add,
    )
    rsq = pool.tile([128, G], F32, tag="xrsq")
    _emit_rsqrt(nc, pool, rsq, ss, G, tag="xfr")
    # scs = min(MAX_NORM * rsqrt(ss), 1) / 2^EXP_S
    scs = pool.tile([128, G], F32, tag="xscs")
    nc.vector.tensor_scalar(
        out=scs[:, :], in0=rsq[:, :],
        scalar1=MAX_NORM / (1 << EXP_S), scalar2=1.0 / (1 << EXP_S),
        op0=AL.mult, op1=AL.min,
    )
    As = pool.tile([128, G * 36], F32, tag="xAs")
    for g in range(G):
        nc.vector.tensor_scalar(
            out=As[:, g * 36 : (g + 1) * 36],
            in0=S[:, g * 36 : (g + 1) * 36],
            scalar1=scs[:, g : g + 1], scalar2=None, op0=AL.mult,
        )

    def bprod(out_tile, left, right, tag):
        """out = left @ right per (token, g). TensorTensor caps at 3 free
        dims, so one broadcast multiply per g into TMP (i, j, l layout),
        then a single segmented reduce over innermost l."""
        TMP = pool.tile([128, G * 216], F32, tag="xTMP")
        for g in range(G):
            lv = (
                left[:, g * 36 : (g + 1) * 36]
                .rearrange("p (i l) -> p i l", i=6)
                .unsqueeze(2)
                .broadcast_to((128, 6, 6, 6))          # p i j l
            )
            rv = (
                right[:, g * 36 : (g + 1) * 36]
                .rearrange("p (l j) -> p l j", l=6)
                .unsqueeze(1)
                .broadcast_to((128, 6, 6, 6))          # p i l j
                .transpose([0, 1, 3, 2])               # p i j l
            )
            tmp_v = TMP[:, g * 216 : (g + 1) * 216].rearrange(
                "p (i j l) -> p i j l", i=6, j=6, l=6
            )
            nc.vector.tensor_tensor(out=tmp_v, in0=lv, in1=rv, op=AL.mult)
        nc.vector.tensor_reduce(
            out=out_tile[:, :],
            in_=TMP[:, :].rearrange("p (q l) -> p q l", l=6),
            axis=AX.X,
            op=AL.add,
        )

    def diag_add_one(t):
        dv = t[:, :].rearrange("p (g a) -> p g a", g=G)[:, :, 0:36:7]
        nc.scalar.add(dv, dv, 1.0)

    # Horner: M = I + As/6; for k=5..2: M = I + (As@M)/k; E = I + As@M
    M = pool.tile([128, G * 36], F32, tag="xM6")
    nc.vector.tensor_scalar(
        out=M[:, :], in0=As[:, :], scalar1=1.0 / 6.0, scalar2=None, op0=AL.mult
    )
    diag_add_one(M)
    for k in (5, 4, 3, 2):
        Pt = pool.tile([128, G * 36], F32, tag="xP")
        bprod(Pt, As, M, tag=f"h{k}")
        M = pool.tile([128, G * 36], F32, tag=f"xM{k}")
        nc.vector.tensor_scalar(
            out=M[:, :], in0=Pt[:, :], scalar1=1.0 / k, scalar2=None, op0=AL.mult
        )
        diag_add_one(M)
    E = pool.tile([128, G * 36], F32, tag="xE0")
    bprod(E, As, M, tag="e0")
    diag_add_one(E)
    for s in range(EXP_S):
        E2 = pool.tile([128, G * 36], F32, tag=f"xE{s+1}")
        bprod(E2, E, E, tag=f"sq{s}")
        E = E2
    return E


def _build(emit_pb, emit_gb):
    nc = bacc.Bacc("TRN2", target_bir_lowering=False, debug=False)

    th = nc.dram_tensor("th", [BLOC, C, D_BB], F32R, kind="ExternalInput")
    pwT = nc.dram_tensor("pwT", [D_BB, D], F32R, kind="ExternalInput")
    wgT = nc.dram_tensor("wgT", [C, D, D], F32R, kind="ExternalInput")
    wvT = nc.dram_tensor("wvT", [C, D, D], F32R, kind="ExternalInput")
    woT = nc.dram_tensor("woT", [C, D, 36], F32, kind="ExternalInput")
    pb = bg = bv = None
    if emit_pb:
        pb = nc.dram_tensor("pb", [1, D], F32R, kind="ExternalInput")
    if emit_gb:
        bg = nc.dram_tensor("bg", [C, D], F32R, kind="ExternalInput")
        bv = nc.dram_tensor("bv", [C, D], F32R, kind="ExternalInput")
    out = nc.dram_tensor("out", [C, BLOC, 36], F32, kind="ExternalOutput")

    with tile.TileContext(nc) as tc, ExitStack() as ctx:
        singles = ctx.enter_context(tc.tile_pool(name="singles", bufs=1))
        cam = ctx.enter_context(tc.tile_pool(name="cam", bufs=2))
        tnat = ctx.enter_context(tc.tile_pool(name="tnat", bufs=5))
        tkp = ctx.enter_context(tc.tile_pool(name="tkp", bufs=3))
        work = ctx.enter_context(tc.tile_pool(name="work", bufs=2))
        xw = ctx.enter_context(tc.tile_pool(name="xw", bufs=2))
        tr_ps = ctx.enter_context(tc.tile_pool(name="tr_ps", bufs=2, space="PSUM"))
        mm_ps = ctx.enter_context(tc.tile_pool(name="mm_ps", bufs=4, space="PSUM"))

        # memset cannot emit float32r directly; stage in f32 and cast-copy.
        identF = singles.tile([128, 128], F32)
        make_identity(nc, identF)
        identR = singles.tile([128, 128], F32R)
        nc.vector.tensor_copy(identR, identF)
        pw_s = singles.tile([128, KE, D], F32R)
        nc.sync.dma_start(pw_s, pwT.ap().rearrange("(k p) d -> p k d", p=128))
        onesF = singles.tile([1, TOK], F32)
        nc.vector.memset(onesF, 1.0)
        ones128 = singles.tile([1, 128], F32R)
        nc.vector.tensor_copy(ones128, onesF[:, :128])
        ones512 = singles.tile([1, TOK], F32R)
        nc.vector.tensor_copy(ones512, onesF)
        pb_s = None
        if emit_pb:
            pb_s = singles.tile([1, D], F32R)
            nc.sync.dma_start(pb_s, pb.ap())

        for c in range(C):
            wg_s = cam.tile([128, KD, D], F32R, tag="wg")
            nc.sync.dma_start(wg_s, wgT.ap()[c].rearrange("(k p) f -> p k f", p=128))
            wv_s = cam.tile([128, KD, D], F32R, tag="wv")
            nc.sync.dma_start(wv_s, wvT.ap()[c].rearrange("(k p) f -> p k f", p=128))
            wo_s = cam.tile([128, KD, 36], F32, tag="wo")
            nc.sync.dma_start(wo_s, woT.ap()[c].rearrange("(k p) o -> p k o", p=128))
            bg_s = bv_s = None
            if emit_gb:
                bg_s = cam.tile([1, D], F32R, tag="bg")
                nc.sync.dma_start(bg_s, bg.ap()[c : c + 1, :])
                bv_s = cam.tile([1, D], F32R, tag="bv")
                nc.sync.dma_start(bv_s, bv.ap()[c : c + 1, :])

            for t0 in range(0, BLOC, TOK):
                tn = []
                for g in range(G):
                    t = tnat.tile([128, D_BB], F32R, tag="tn")
                    nc.sync.dma_start(t, th.ap()[t0 + g * 128 : t0 + (g + 1) * 128, c, :])
                    tn.append(t)

                # ---- mm1: reg[t, d] = thumb @ pwT  (PE transpose per k-chunk)
                reg_ps = [
                    mm_ps.tile([128, D], F32, tag="mm", name=f"reg{g}")
                    for g in range(G)
                ]
                for k in range(KE):
                    tr = tr_ps.tile([128, TOK], F32R, tag="tr")
                    for g in range(G):
                        nc.tensor.transpose(
                            tr[:, g * 128 : (g + 1) * 128],
                            tn[g][:, k * 128 : (k + 1) * 128],
                            identR,
                        )
                    tk = tkp.tile([128, TOK], F32R, tag="tk")
                    nc.scalar.copy(tk, tr)
                    for g in range(G):
                        nc.tensor.matmul(
                            reg_ps[g],
                            tk[:, g * 128 : (g + 1) * 128],
                            pw_s[:, k, :],
                            start=(k == 0),
                            stop=(k == KE - 1 and not emit_pb),
                        )
                if emit_pb:
                    for g in range(G):
                        nc.tensor.matmul(
                            reg_ps[g], ones128, pb_s, start=False, stop=True
                        )

                # ---- LayerNorm stats + y = (reg - mu) * rstd   (fp32r)
                mv = work.tile([128, G, 2], F32, tag="mv")
                for g in range(G):
                    st = work.tile([128, 6], F32, tag="bst")
                    nc.vector.bn_stats(out=st[:, :], in_=reg_ps[g])
                    nc.vector.bn_aggr(out=mv[:, g, :], in_=st[:, :])
                vpe = work.tile([128, G], F32, tag="vpe")
                nc.vector.tensor_scalar(
                    out=vpe[:, :], in0=mv[:, :, 1], scalar1=LN_EPS, scalar2=None,
                    op0=AL.add,
                )
                rstd = work.tile([128, G], F32, tag="rstd")
                _emit_rsqrt(nc, work, rstd, vpe, G, tag="ln")
                y = work.tile([128, G, D], F32R, tag="y")
                for g in range(G):
                    nc.vector.tensor_scalar(
                        out=y[:, g, :], in0=reg_ps[g],
                        scalar1=mv[:, g, 0:1], scalar2=rstd[:, g : g + 1],
                        op0=AL.subtract, op1=AL.mult,
                    )

                # ---- transpose y -> yT [d, t]
                yT = work.tile([128, KD, TOK], F32R, tag="yT")
                for kd in range(KD):
                    tr = tr_ps.tile([128, TOK], F32R, tag="tr")
                    for g in range(G):
                        nc.tensor.transpose(
                            tr[:, g * 128 : (g + 1) * 128],
                            y[:, g, kd * 128 : (kd + 1) * 128],
                            identR,
                        )
                    nc.scalar.copy(yT[:, kd, :], tr)

                # ---- mm2 gate/val + silu + h
                h = work.tile([128, KD, TOK], F32, tag="h")
                for mf in range(KD):
                    g_ps = mm_ps.tile([128, TOK], F32, tag="mm")
                    for kd in range(KD):
                        nc.tensor.matmul(
                            g_ps,
                            wg_s[:, kd, mf * 128 : (mf + 1) * 128],
                            yT[:, kd, :],
                            start=(kd == 0),
                            stop=(kd == KD - 1 and not emit_gb),
                        )
                    if emit_gb:
                        nc.tensor.matmul(
                            g_ps, bg_s[:, mf * 128 : (mf + 1) * 128], ones512,
                            start=False, stop=True,
                        )
                    v_ps = mm_ps.tile([128, TOK], F32, tag="mm")
                    for kd in range(KD):
                        nc.tensor.matmul(
                            v_ps,
                            wv_s[:, kd, mf * 128 : (mf + 1) * 128],
                            yT[:, kd, :],
                            start=(kd == 0),
                            stop=(kd == KD - 1 and not emit_gb),
                        )
                    if emit_gb:
                        nc.tensor.matmul(
                            v_ps, bv_s[:, mf * 128 : (mf + 1) * 128], ones512,
                            start=False, stop=True,
                        )
                    sg = work.tile([128, TOK], F32, tag="sg")
                    nc.scalar.activation(sg, g_ps, AF.Silu)
                    nc.vector.tensor_tensor(
                        out=h[:, mf, :], in0=sg, in1=v_ps, op=AL.mult
                    )

                # ---- mm3: A0[t, 36] = h.T @ woT  (h is the stationary operand)
                a0_ps = tr_ps.tile([128, G, 36], F32, tag="a0")
                for g in range(G):
                    for kf in range(KD):
                        nc.tensor.matmul(
                            a0_ps[:, g, :],
                            h[:, kf, g * 128 : (g + 1) * 128],
                            wo_s[:, kf, :],
                            start=(kf == 0),
                            stop=(kf == KD - 1),
                        )
                A0 = xw.tile([128, G * 36], F32, tag="A0")
                nc.scalar.copy(A0, a0_ps[:, :, :].rearrange("p g a -> p (g a)"))

                E = _emit_expm(nc, xw, A0)
                nc.sync.dma_start(
                    out.ap()[c, t0 : t0 + TOK, :].rearrange(
                        "(g p) a -> p g a", p=128
                    ),
                    E[:, :].rearrange("p (g a) -> p g a", g=G),
                )

    nc.compile()
    return nc


def kernel(**inputs):
    global last_results, last_in_maps
    thumb = np.ascontiguousarray(np.asarray(inputs["thumbnails"], dtype=np.float32))
    proj_w = np.asarray(inputs["proj_w"], dtype=np.float32)
    proj_b = np.asarray(inputs["proj_b"], dtype=np.float32)
    gamma = np.asarray(inputs["gamma"], dtype=np.float32)
    beta = np.asarray(inputs["beta"], dtype=np.float32)
    w_gate = np.asarray(inputs["w_gate"], dtype=np.float32)
    w_val = np.asarray(inputs["w_val"], dtype=np.float32)
    w_out = np.asarray(inputs["w_out"], dtype=np.float32)

    # host-side weight prep: fold gamma into the SwiGLU weights, beta into
    # rank-1 biases, pre-transpose everything for the PE's lhsT convention.
    pwT = np.ascontiguousarray(proj_w.T)                        # [D_BB, D]
    wgT = np.ascontiguousarray((w_gate * gamma[:, None, :]).transpose(0, 2, 1))
    wvT = np.ascontiguousarray((w_val * gamma[:, None, :]).transpose(0, 2, 1))
    woT = np.ascontiguousarray(w_out.transpose(0, 2, 1))        # [C, D, 36]
    bg = np.einsum("cfd,cd->cf", w_gate, beta).astype(np.float32)
    bv = np.einsum("cfd,cd->cf", w_val, beta).astype(np.float32)

    emit_pb = bool(np.any(proj_b))
    emit_gb = bool(np.any(bg) or np.any(bv))

    key = (emit_pb, emit_gb)
    if key not in _BUILD_CACHE:
        _BUILD_CACHE[key] = _build(emit_pb, emit_gb)
    nc = _BUILD_CACHE[key]

    shared = {"pwT": pwT, "wgT": wgT, "wvT": wvT, "woT": woT}
    if emit_pb:
        shared["pb"] = proj_b.reshape(1, D)
    if emit_gb:
        shared["bg"] = bg
        shared["bv"] = bv
    in_maps = []
    for i in range(N_CORES):
        m = dict(shared)
        m["th"] = thumb[i * BLOC : (i + 1) * BLOC]
        in_maps.append(m)

    last_in_maps = in_maps
    trace = bool(int(os.environ.get("KERNEL_TRACE", "0")))
    last_results = run_bass_kernel_spmd(
        nc, in_maps, core_ids=list(range(N_CORES)), trace=trace
    )
    parts = [r["out"] for r in last_results.results]            # [C, BLOC, 36]
    full = np.concatenate(parts, axis=1)                        # [C, B, 36]
    return full.reshape(C, B, 6, 6)
